# revision 3
# baseline (speedup 1.0000x reference)
"""Trainium2 Bass kernel for CoveragePlannerNet (GNN message passing).

Computation (per batch b of 512):
  h  = relu(x @ W1T) ; h = relu(h @ W2T)          # per-agent MLP  [256, 8]
  z0 = h^T                                        # [8, 256]
  z1 = z0 @ S_b ; z2 = z1 @ S_b                   # K=3 graph-shift taps
  y  = relu(sum_k Hgf[:,k,:] @ zk + bgf)          # [16, 256]
  act= y^T @ Wa^T + ba                            # [256, 5]

Data-parallel over batch across 8 NeuronCores (64 batches/core).
DMA-bound by S (512*256*256*4 = 134 MB total, ~16.8 MB/core).

Per-core layout strategy (groups of 16 batches, octets of 8):
  - x is host-transposed/packed so the agent MLP runs as [128,128]
    matmuls with block-diagonal weights (4 agent-chunks per pass).
  - Graph shifts keep S as the *moving* operand (no big LDWEIGHTS):
    lhsT = [128, 8] slices of the transposed-tap tiles.
  - Tap transposes are batched (8 batches per [64,128] PE transpose).
  - Graph filter + action head run 8 batches per matmul via
    block-diagonal host-packed weights.
"""

import os

os.environ.setdefault("JAX_COMPILATION_CACHE_DIR", "/tmp/jaxcache_covplan")

import numpy as np

import concourse.bass as bass
import concourse.mybir as mybir
import concourse.tile as tile
from concourse.bass_utils import run_bass_kernel_spmd

F32 = mybir.dt.float32
RELU = mybir.ActivationFunctionType.Relu
IDENT = mybir.ActivationFunctionType.Identity

NCORES = 8
B, N, FIN = 512, 256, 32
H1, G, FOUT, K, ACT = 16, 8, 16, 3, 5
BC = B // NCORES            # 64 batches per core
GRP = 16                    # batches per group
NGRP = BC // GRP


def _split_waits(nc, limit=1):
    """walrus here accepts at most `limit` sem-waits per instruction; move
    excess waits onto preceding NOPs on the same engine/queue."""
    for bb in nc.main_func.blocks:
        new_insts = []
        for ins in bb.instructions:
            si = ins.sync_info
            if si is not None and si.on_wait and len(si.on_wait) > limit:
                waits = list(si.on_wait)
                si.on_wait = waits[-limit:]
                head = waits[:-limit]
                for i in range(0, len(head), limit):
                    nop = mybir.InstNoOp(
                        name=f"WSPLIT-{nc.next_id()}", ins=[], outs=[]
                    )
                    nop.engine = ins.engine
                    nop.sync_info = mybir.SyncInfo(
                        on_wait=head[i : i + limit], on_update=[]
                    )
                    new_insts.append(nop)
            new_insts.append(ins)
        bb.instructions[:] = new_insts


def _build_program():
    nc = bass.Bass()

    xTs = nc.dram_tensor("xTs", [BC // 2 * 128, 128], F32, kind="ExternalInput")
    S_d = nc.dram_tensor("S", [BC * 256, 256], F32, kind="ExternalInput")
    bdW1T = nc.dram_tensor("bdW1T", [128, 64], F32, kind="ExternalInput")
    b1t = nc.dram_tensor("b1t", [64, 1], F32, kind="ExternalInput")
    bdW2T = nc.dram_tensor("bdW2T", [64, 32], F32, kind="ExternalInput")
    b2t = nc.dram_tensor("b2t", [32, 1], F32, kind="ExternalInput")
    bdH0 = nc.dram_tensor("bdH0", [64, 128], F32, kind="ExternalInput")
    bdH1 = nc.dram_tensor("bdH1", [64, 128], F32, kind="ExternalInput")
    bdH2 = nc.dram_tensor("bdH2", [64, 128], F32, kind="ExternalInput")
    bgft = nc.dram_tensor("bgft", [128, 1], F32, kind="ExternalInput")
    bdWaT = nc.dram_tensor("bdWaT", [128, 40], F32, kind="ExternalInput")
    bat = nc.dram_tensor("bat", [40, 1], F32, kind="ExternalInput")
    ident_d = nc.dram_tensor("ident", [128, 128], F32, kind="ExternalInput")
    outT = nc.dram_tensor("outT", [BC * ACT, 256], F32, kind="ExternalOutput")

    with tile.TileContext(nc) as tc:
        with (
            tc.tile_pool(name="singles", bufs=1) as singles,
            tc.tile_pool(name="sx", bufs=3) as sx,
            tc.tile_pool(name="ss", bufs=2) as ssp,
            tc.tile_pool(name="sz", bufs=2) as szp,
            tc.tile_pool(name="sy", bufs=2) as syp,
            tc.tile_pool(name="sa", bufs=3) as sap,
            tc.tile_pool(name="ph1", bufs=1, space="PSUM") as ph1,
            tc.tile_pool(name="pz0", bufs=1, space="PSUM") as pz0,
            tc.tile_pool(name="pzp", bufs=2, space="PSUM") as pzp,
            tc.tile_pool(name="ptr", bufs=2, space="PSUM") as ptr,
            tc.tile_pool(name="py", bufs=1, space="PSUM") as pyp,
            tc.tile_pool(name="pa", bufs=1, space="PSUM") as pap,
        ):
            # ---- load constants ----
            t_w1 = singles.tile([128, 64], F32)
            t_b1 = singles.tile([64, 1], F32)
            t_w2 = singles.tile([64, 32], F32)
            t_b2 = singles.tile([32, 1], F32)
            t_h = [singles.tile([64, 128], F32, name=f"th{k}", tag=f"th{k}") for k in range(3)]
            t_bgf = singles.tile([128, 1], F32)
            t_wa = singles.tile([128, 40], F32)
            t_ba = singles.tile([40, 1], F32)
            t_id = singles.tile([128, 128], F32)
            for t, d in [
                (t_w1, bdW1T), (t_b1, b1t), (t_w2, bdW2T), (t_b2, b2t),
                (t_h[0], bdH0), (t_h[1], bdH1), (t_h[2], bdH2), (t_bgf, bgft),
                (t_wa, bdWaT), (t_ba, bat), (t_id, ident_d),
            ]:
                nc.sync.dma_start(out=t[:], in_=d[:])

            for g in range(NGRP):
                b0 = g * GRP

                # ---- A: agent MLP, 2 batches (4 agent-chunks) per pass ----
                # z0g[o][8*bg8+g', n] = z0 for octet o, local batch bg8
                z0g = [szp.tile([64, 256], F32, name=f"z0g{o}", tag=f"z0g{o}") for o in range(2)]
                for p in range(GRP // 2):
                    pg = (b0 + 2 * p) // 2
                    xt = sx.tile([128, 128], F32, tag="xt")
                    nc.sync.dma_start(
                        out=xt[:], in_=xTs[pg * 128 : (pg + 1) * 128, :]
                    )
                    p_h1 = ph1.tile([64, 128], F32, tag="ph1")
                    nc.tensor.matmul(p_h1[:], t_w1[:], xt[:], start=True, stop=True)
                    s_h1 = sx.tile([64, 128], F32, tag="h1")
                    nc.scalar.activation(s_h1[:], p_h1[:], RELU, bias=t_b1[:])
                    p_z0 = pz0.tile([32, 128], F32, tag="pz0")
                    nc.tensor.matmul(p_z0[:], t_w2[:], s_h1[:], start=True, stop=True)
                    zst = sx.tile([32, 128], F32, tag="zst")
                    nc.scalar.activation(zst[:], p_z0[:], RELU, bias=t_b2[:])
                    # scatter rows (h, b_off, g') -> z0g[o][(bg8, g'), n-half]
                    o, q = p // 4, p % 4
                    for h in range(2):
                        nc.sync.dma_start(
                            out=z0g[o][16 * q : 16 * q + 16, 128 * h : 128 * h + 128],
                            in_=zst[16 * h : 16 * h + 16, :],
                        )

                # ---- T0: batched transpose z0g -> z0T ----
                # z0T[:, 128c + 64o + 8bg8 + g'] = z0[b, g', 128c + n']
                z0T = szp.tile([128, 256], F32, tag="z0T")
                for o in range(2):
                    for c in range(2):
                        p_t = ptr.tile([128, 64], F32, tag="ptr")
                        nc.tensor.transpose(
                            p_t[:], z0g[o][:, 128 * c : 128 * c + 128],
                            t_id[0:64, 0:64],
                        )
                        nc.vector.tensor_copy(
                            z0T[:, 128 * c + 64 * o : 128 * c + 64 * o + 64], p_t[:]
                        )

                def zcol(bg, c):
                    return 128 * c + 64 * (bg // 8) + 8 * (bg % 8)

                # ---- C1: shift1 (z1 = z0 @ S), pair-packed psum ----
                ss = ssp.tile([128, GRP * 512], F32, tag="ss")
                z1p = []
                for pr in range(GRP // 2):
                    p_zp = pzp.tile([64, 256], F32, tag="pzp")
                    for i in range(2):
                        bg = 2 * pr + i
                        b = b0 + bg
                        for c in range(2):
                            nc.sync.dma_start(
                                out=ss[:, 512 * bg + 256 * c : 512 * bg + 256 * (c + 1)],
                                in_=S_d[256 * b + 128 * c : 256 * b + 128 * (c + 1), :],
                            )
                            nc.tensor.matmul(
                                p_zp[32 * i : 32 * i + 8, :],
                                z0T[:, zcol(bg, c) : zcol(bg, c) + 8],
                                ss[:, 512 * bg + 256 * c : 512 * bg + 256 * (c + 1)],
                                start=(c == 0),
                                stop=(c == 1),
                            )
                    sq = szp.tile([64, 256], F32, tag="z1p")
                    nc.vector.tensor_copy(sq[:], p_zp[:])
                    z1p.append(sq)

                # gather sparse pair rows -> dense octet tiles
                z1g = [szp.tile([64, 256], F32, name=f"z1g{o}", tag=f"z1g{o}") for o in range(2)]
                for pr in range(GRP // 2):
                    for i in range(2):
                        bg = 2 * pr + i
                        nc.sync.dma_start(
                            out=z1g[bg // 8][8 * (bg % 8) : 8 * (bg % 8) + 8, :],
                            in_=z1p[pr][32 * i : 32 * i + 8, :],
                        )

                # ---- T1: batched transpose z1g -> z1T ----
                z1T = szp.tile([128, 256], F32, tag="z1T")
                for o in range(2):
                    for c in range(2):
                        p_t = ptr.tile([128, 64], F32, tag="ptr")
                        nc.tensor.transpose(
                            p_t[:], z1g[o][:, 128 * c : 128 * c + 128],
                            t_id[0:64, 0:64],
                        )
                        nc.vector.tensor_copy(
                            z1T[:, 128 * c + 64 * o : 128 * c + 64 * o + 64], p_t[:]
                        )

                # ---- C2: shift2 (z2 = z1 @ S), S already resident ----
                z2g = [szp.tile([64, 256], F32, name=f"z2g{o}", tag=f"z2g{o}") for o in range(2)]
                for pr in range(GRP // 2):
                    p_zp = pzp.tile([64, 256], F32, tag="pzp")
                    for i in range(2):
                        bg = 2 * pr + i
                        for c in range(2):
                            nc.tensor.matmul(
                                p_zp[32 * i : 32 * i + 8, :],
                                z1T[:, zcol(bg, c) : zcol(bg, c) + 8],
                                ss[:, 512 * bg + 256 * c : 512 * bg + 256 * (c + 1)],
                                start=(c == 0),
                                stop=(c == 1),
                            )
                    sq = szp.tile([64, 256], F32, tag="z2p")
                    nc.vector.tensor_copy(sq[:], p_zp[:])
                    for i in range(2):
                        bg = 2 * pr + i
                        nc.sync.dma_start(
                            out=z2g[bg // 8][8 * (bg % 8) : 8 * (bg % 8) + 8, :],
                            in_=sq[32 * i : 32 * i + 8, :],
                        )

                # ---- D+E: graph filter + action head, 8 batches per pass ----
                for o in range(2):
                    p_y = pyp.tile([128, 256], F32, tag="py")
                    taps = (z0g[o], z1g[o], z2g[o])
                    for k in range(3):
                        nc.tensor.matmul(
                            p_y[:], t_h[k][:], taps[k][:],
                            start=(k == 0), stop=(k == 2),
                        )
                    s_y = syp.tile([128, 256], F32, tag="sy")
                    nc.scalar.activation(s_y[:], p_y[:], RELU, bias=t_bgf[:])
                    p_a = pap.tile([40, 256], F32, tag="pa")
                    nc.tensor.matmul(p_a[:], t_wa[:], s_y[:], start=True, stop=True)
                    s_a = sap.tile([40, 256], F32, tag="sa")
                    nc.scalar.activation(s_a[:], p_a[:], IDENT, bias=t_ba[:])
                    r0 = (b0 + 8 * o) * ACT
                    nc.sync.dma_start(out=outT[r0 : r0 + 40, :], in_=s_a[:])

    _split_waits(nc)
    return nc


def _prep_params(W1, b1, W2, b2, Hgf, bgf, Wa, ba):
    f32 = np.float32
    bdW1T = np.zeros((128, 64), f32)
    for c in range(4):
        bdW1T[32 * c : 32 * c + 32, 16 * c : 16 * c + 16] = W1.T
    bdW2T = np.zeros((64, 32), f32)
    for c in range(4):
        bdW2T[16 * c : 16 * c + 16, 8 * c : 8 * c + 8] = W2.T
    # filter weights: bdH[k][8i+g', 16i+f] = Hgf[f, k, g']  (8-batch block diag)
    bdH = []
    for k in range(3):
        E = np.zeros((64, 128), f32)
        for i in range(8):
            E[8 * i : 8 * i + 8, 16 * i : 16 * i + 16] = Hgf[:, k, :].T
        bdH.append(E)
    bdWaT = np.zeros((128, 40), f32)
    for i in range(8):
        bdWaT[16 * i : 16 * i + 16, 5 * i : 5 * i + 5] = Wa.T
    return {
        "bdW1T": bdW1T,
        "b1t": np.ascontiguousarray(np.tile(b1, 4)[:, None].astype(f32)),
        "bdW2T": bdW2T,
        "b2t": np.ascontiguousarray(np.tile(b2, 4)[:, None].astype(f32)),
        "bdH0": bdH[0], "bdH1": bdH[1], "bdH2": bdH[2],
        "bgft": np.ascontiguousarray(np.tile(bgf, 8)[:, None].astype(f32)),
        "bdWaT": bdWaT,
        "bat": np.ascontiguousarray(np.tile(ba, 8)[:, None].astype(f32)),
        "ident": np.eye(128, dtype=f32),
    }


_CACHE = {}


def kernel(x, S, W1, b1, W2, b2, Hgf, bgf, Wa, ba):
    x = np.asarray(x, np.float32)
    S = np.asarray(S, np.float32)
    if "nc" not in _CACHE:
        _CACHE["nc"] = _build_program()
    nc = _CACHE["nc"]

    params = _prep_params(
        np.asarray(W1, np.float32), np.asarray(b1, np.float32),
        np.asarray(W2, np.float32), np.asarray(b2, np.float32),
        np.asarray(Hgf, np.float32), np.asarray(bgf, np.float32),
        np.asarray(Wa, np.float32), np.asarray(ba, np.float32),
    )

    in_maps = []
    for core in range(NCORES):
        bsl = slice(core * BC, (core + 1) * BC)
        xs = x[bsl]                                   # [BC, 256, 32]
        # pack rows (pair, half, b_off, f) x cols n'
        xr = xs.reshape(BC // 2, 2, 2, 128, FIN)      # [p, b_off, h, n', f]
        xt = np.ascontiguousarray(xr.transpose(0, 2, 1, 4, 3))  # [p, h, b_off, f, n']
        m = {
            "xTs": xt.reshape(BC // 2 * 128, 128),
            "S": np.ascontiguousarray(S[bsl]).reshape(BC * 256, 256),
        }
        m.update(params)
        in_maps.append(m)

    res = run_bass_kernel_spmd(nc, in_maps, list(range(NCORES)))
    outs = []
    for core in range(NCORES):
        o = res.results[core]["outT"].reshape(BC, ACT, 256)
        outs.append(o.transpose(0, 2, 1))             # [BC, 256, 5]
    return np.ascontiguousarray(np.concatenate(outs, axis=0))
